# revision 6
# baseline (speedup 1.0000x reference)
# kernel.py — Bidirectional masked-GRU-with-predictor on 8 Trainium2 NeuronCores.
#
# Problem (reference.py): B=128, T=1024, H=512
#   per step, per direction:
#     x_in = where(mask, predictor(h), x)            predictor: Linear(H,H)->ReLU->Linear(H,1)->Tanh
#     h    = GRUCell(h, x_in)                        PyTorch gate order (r, z, n)
#   output [B, T, 2H] = concat(fwd hidden states, time-reversed bwd hidden states)
#
# Sharding: 8 cores = 2 directions x 4 batch groups of 32.  All cores run the
# SAME Bass program; per-core data differs (bwd cores get time-reversed x/mask
# and their outputs are flipped back on the host).
#
# On-core layout ("feature-major, chunk-in-free"):
#   h^T kept as [128 partitions = feature%128, (j,b)] where j = feature//128 (4 chunks),
#   b = local batch (32).  Big matmul: stationary = W^T 128x128 blocks (fp16, FWL),
#   moving = h chunks; gates + predictor-hidden land in PSUM feature-major, so the
#   new h is produced directly in the layout the next step's matmul consumes.
#   fp16 matmul inputs + fp32 PSUM accumulate + fp32 vector math:
#   measured emulation error vs fp32 reference: ~7e-4 of output absmax.

import numpy as np

B, T, H = 128, 1024, 512
NCORES = 8
BL = B // 4          # 32: batch per core (4 groups x 2 directions)
KC = H // 128        # 4 contraction chunks
MC = (3 * H + H) // 128  # 16 output chunks (w_hh 12 + p_w1 4)
U_DEF = 16           # time steps per For_i iteration

_cache = {}


def _build_program(t_steps=T, u_steps=U_DEF, bl=BL, n_cores=NCORES):
    import concourse.bacc as bacc
    import concourse.bass as bass
    import concourse.tile as tile
    from concourse.tile import add_dep_helper
    from concourse import mybir

    f16 = mybir.dt.float16
    f32 = mybir.dt.float32

    nc = bacc.Bacc(
        "TRN2",
        target_bir_lowering=False,
        debug=False,
        enable_asserts=False,
        num_devices=n_cores,
    )

    # ---- DRAM tensors (per-core data; same names on every core) ----
    d_wt = nc.dram_tensor("wt", [128, MC * KC * 128], f16, kind="ExternalInput").ap()
    d_gi = nc.dram_tensor("gilhs", [2, 12 * 128], f16, kind="ExternalInput").ap()
    d_bc = nc.dram_tensor("bcols", [4, 5 * 128], f16, kind="ExternalInput").ap()
    d_e4 = nc.dram_tensor("e4", [4, KC * bl], f16, kind="ExternalInput").ap()
    d_pw2 = nc.dram_tensor("pw2t", [128, KC], f16, kind="ExternalInput").ap()
    d_pb2 = nc.dram_tensor("pb2", [1, 1], f32, kind="ExternalInput").ap()
    d_a = nc.dram_tensor("a_arr", [t_steps, bl], f16, kind="ExternalInput").ap()
    d_m = nc.dram_tensor("m_arr", [t_steps, bl], f16, kind="ExternalInput").ap()
    d_out = nc.dram_tensor(
        "outl", [t_steps, 128, KC, bl], f16, kind="ExternalOutput"
    ).ap()

    Relu = mybir.ActivationFunctionType.Relu
    Tanh = mybir.ActivationFunctionType.Tanh
    Sigmoid = mybir.ActivationFunctionType.Sigmoid

    with tile.TileContext(nc) as tc:
        import contextlib

        with contextlib.ExitStack() as ctx:
            consts = ctx.enter_context(tc.tile_pool(name="consts", bufs=1))
            psum = ctx.enter_context(tc.tile_pool(name="psum", bufs=1, space="PSUM"))
            work = ctx.enter_context(tc.tile_pool(name="work", bufs=2))
            io = ctx.enter_context(tc.tile_pool(name="io", bufs=2))

            # ---- constant preload ----
            WT = consts.tile([128, MC * KC * 128], f16, tag="WT")
            GIL = consts.tile([2, 12 * 128], f16, tag="GIL")
            BC = consts.tile([4, 5 * 128], f16, tag="BC")
            E4 = consts.tile([4, KC * bl], f16, tag="E4")
            PW2 = consts.tile([128, KC], f16, tag="PW2")
            PB2 = consts.tile([1, 1], f32, tag="PB2")
            for dst, src in (
                (WT, d_wt), (GIL, d_gi), (BC, d_bc),
                (E4, d_e4), (PW2, d_pw2), (PB2, d_pb2),
            ):
                nc.sync.dma_start(out=dst, in_=src)

            # persistent ping-pong hidden state, fp16, [128, (j,b)]
            h0 = consts.tile([128, KC * bl], f16, tag="h0")
            h1 = consts.tile([128, KC * bl], f16, tag="h1")
            nc.vector.memset(h0, 0.0)
            nc.vector.memset(h1, 0.0)
            h_tiles = [h0, h1]

            # persistent PSUM accumulators (single-buffered; readers drain early)
            G_r = psum.tile([128, KC * bl], f32, tag="G_r")
            G_z = psum.tile([128, KC * bl], f32, tag="G_z")
            G_n = psum.tile([128, KC * bl], f32, tag="G_n")
            PHp = psum.tile([128, KC * bl], f32, tag="PH")
            GIN = psum.tile([128, KC * bl], f32, tag="GIN")
            PRD = psum.tile([1, bl], f32, tag="PRD")

            def w_block(m, k):
                bi = m * KC + k
                return WT[:, bi * 128:(bi + 1) * 128]

            def emit_region(g_idx, region, h_cur, has_gi=False,
                            j_range=(0, KC), emit_bias=True):
                # bias matmul opens the accumulation (start=True covers the
                # whole region), then j_range m-chunks x 4 k-chunks of W
                # blocks.  Returns (first, last) for PE-order chaining.
                first = last = None
                if emit_bias:
                    first = last = nc.tensor.matmul(
                        region, BC[:, g_idx * 128:(g_idx + 1) * 128], E4,
                        start=True, stop=False, skip_group_check=True,
                    )
                base_m = g_idx * KC if g_idx < 3 else 12
                for j in range(*j_range):
                    m = base_m + j
                    for k in range(KC):
                        last = nc.tensor.matmul(
                            region[:, j * bl:(j + 1) * bl],
                            w_block(m, k),
                            h_cur[:, k * bl:(k + 1) * bl],
                            start=False,
                            stop=(not has_gi and k == KC - 1),
                            skip_group_check=True,
                        )
                        if first is None:
                            first = last
                return first, last

            def emit_gi(g_idx, region, gi_rhs):
                # K=2 rank-1 matmuls: region[:, j] += w_ih_g[j] (x) x_in
                first = last = None
                for j in range(KC):
                    gj = g_idx * KC + j
                    last = nc.tensor.matmul(
                        region[:, j * bl:(j + 1) * bl],
                        GIL[:, gj * 128:(gj + 1) * 128],
                        gi_rhs,
                        start=False, stop=True, skip_group_check=True,
                    )
                    if first is None:
                        first = last
                return first, last

            def pe_order(a_first, b_last):
                # force PE issue order: a runs after b (ordering only)
                add_dep_helper(a_first.ins, b_last.ins, sync=False)

            def step(u, h_cur, h_new, S2, MB, t_dyn):
                gi_rhs = S2[:, u * bl:(u + 1) * bl]
                # PE order: PH, Wr_a, PRD, Wr_b, gi_r, Wn, GIN, Wz, gi_z —
                # the pred chain and gi_r slot in early so the r->n chain
                # (sig_r -> u_n -> pre_n -> tanh_n) starts as soon as
                # possible; z stays last (shortest post-PSUM tail).
                ph_f, ph_l = emit_region(3, PHp, h_cur)
                ra_f, ra_l = emit_region(0, G_r, h_cur, has_gi=True,
                                         j_range=(0, 2))
                pe_order(ra_f, ph_l)
                relu = work.tile([128, KC * bl], f16, tag="relu")
                nc.vector.tensor_scalar_max(relu, PHp, 0.0)
                # --- pred = tanh(p_w2 @ relu + p_b2) ---
                prd_f = prd_l = None
                for k in range(KC):
                    prd_l = nc.tensor.matmul(
                        PRD, PW2[:, k:k + 1], relu[:, k * bl:(k + 1) * bl],
                        start=(k == 0), stop=(k == KC - 1), skip_group_check=True,
                    )
                    if prd_f is None:
                        prd_f = prd_l
                pe_order(prd_f, ra_l)
                pred = work.tile([1, bl], f16, tag="pred")
                nc.scalar.activation(out=pred, in_=PRD, func=Tanh, bias=PB2[:, :])
                # --- x_in = pred*m + x*(1-m):  tmp row of S2 + host a row
                nc.vector.tensor_mul(
                    S2[0:1, u * bl:(u + 1) * bl], pred, MB[0:1, u * bl:(u + 1) * bl]
                )

                rb_f, rb_l = emit_region(0, G_r, h_cur, has_gi=True,
                                         j_range=(2, 4), emit_bias=False)
                pe_order(rb_f, prd_l)
                gir_f, gir_l = emit_gi(0, G_r, gi_rhs)
                pe_order(gir_f, rb_l)
                n_f, n_l = emit_region(2, G_n, h_cur)   # n: gh + b_hh only
                pe_order(n_f, gir_l)
                # GIN = w_ih_n (x) x_in + b_ih_n
                gin_f = nc.tensor.matmul(
                    GIN, BC[:, 4 * 128:5 * 128], E4,
                    start=True, stop=False, skip_group_check=True)
                pe_order(gin_f, n_l)
                _, gin_l = emit_gi(2, GIN, gi_rhs)
                z_f, z_l = emit_region(1, G_z, h_cur, has_gi=True)
                pe_order(z_f, gin_l)
                giz_f, _ = emit_gi(1, G_z, gi_rhs)
                pe_order(giz_f, z_l)

                r_sb = work.tile([128, KC * bl], f16, tag="r_sb")
                nc.scalar.activation(out=r_sb, in_=G_r, func=Sigmoid)

                # n = tanh(GIN + r * G_n)
                u_n = work.tile([128, KC * bl], f32, tag="u_n")
                nc.vector.tensor_mul(u_n, r_sb, G_n)
                pre_n = work.tile([128, KC * bl], f32, tag="pre_n")
                nc.vector.tensor_add(pre_n, u_n, GIN)
                n_sb = work.tile([128, KC * bl], f16, tag="n_sb")
                nc.scalar.activation(out=n_sb, in_=pre_n, func=Tanh)

                z_sb = work.tile([128, KC * bl], f16, tag="z_sb")
                nc.scalar.activation(out=z_sb, in_=G_z, func=Sigmoid)

                # h' = z*h - (z-1)*n  == z*h + (1-z)*n
                t1 = work.tile([128, KC * bl], f16, tag="t1")
                nc.vector.tensor_mul(t1, z_sb, h_cur)
                t2 = work.tile([128, KC * bl], f16, tag="t2")
                nc.vector.scalar_tensor_tensor(
                    out=t2, in0=z_sb, scalar=1.0, in1=n_sb,
                    op0=mybir.AluOpType.subtract, op1=mybir.AluOpType.mult,
                )
                nc.vector.tensor_sub(h_new, t1, t2)

                # stream h' out:  outl[t, p, j, b]
                dst = d_out[bass.ds(t_dyn, 1)].rearrange("o p j b -> (o p) j b")
                nc.sync.dma_start(
                    out=dst, in_=h_new.rearrange("p (j b) -> p j b", b=bl)
                )

            n_blocks = t_steps // u_steps
            with tc.For_i(
                0, n_blocks, 1, hint_engines=(mybir.EngineType.PE,)
            ) as iv:
                S2 = io.tile([2, u_steps * bl], f16, tag="S2")
                MB = io.tile([1, u_steps * bl], f16, tag="MB")
                nc.sync.dma_start(
                    out=S2[1:2, :].rearrange("p (u b) -> p u b", b=bl),
                    in_=d_a[bass.ds(iv * u_steps, u_steps)].unsqueeze(0),
                )
                nc.sync.dma_start(
                    out=MB[0:1, :].rearrange("p (u b) -> p u b", b=bl),
                    in_=d_m[bass.ds(iv * u_steps, u_steps)].unsqueeze(0),
                )
                for u in range(u_steps):
                    step(
                        u,
                        h_tiles[u % 2],
                        h_tiles[(u + 1) % 2],
                        S2,
                        MB,
                        iv * u_steps + u,
                    )

    nc.compile()
    return nc


def _prep_core_inputs(inputs, core, t_steps=T, bl=BL):
    """Build the per-core input map (numpy) for core id `core`."""
    f16 = np.float16
    direction = 0 if core < 4 else 1  # 0 fwd, 1 bwd
    bg = core % 4
    sl = slice(bg * bl, (bg + 1) * bl)

    x = np.asarray(inputs["x"], np.float32)[:, :, 0]      # [B, T]
    msk = np.asarray(inputs["mask"]).astype(np.float32)[:, :, 0]
    pfx = "wf" if direction == 0 else "wb"
    w_ih = np.asarray(inputs[f"{pfx}_ih"], np.float32)[:, 0]   # [3H]
    w_hh = np.asarray(inputs[f"{pfx}_hh"], np.float32)         # [3H, H]
    b_ih = np.asarray(inputs[f"b{pfx[1]}_ih"], np.float32)
    b_hh = np.asarray(inputs[f"b{pfx[1]}_hh"], np.float32)
    p_w1 = np.asarray(inputs["p_w1"], np.float32)
    p_b1 = np.asarray(inputs["p_b1"], np.float32)
    p_w2 = np.asarray(inputs["p_w2"], np.float32)
    p_b2 = np.asarray(inputs["p_b2"], np.float32)

    xs = x[sl].T.copy()      # [T, bl]
    ms = msk[sl].T.copy()
    if direction == 1:
        xs = xs[::-1].copy()
        ms = ms[::-1].copy()
    a_arr = (xs * (1.0 - ms)).astype(f16)
    m_arr = ms.astype(f16)

    W = np.concatenate([w_hh, p_w1], axis=0)             # [2048, 512]
    Wr = W.reshape(MC, 128, KC, 128)                     # [m, c, k, p]
    wt = Wr.transpose(3, 0, 2, 1).reshape(128, MC * KC * 128).astype(f16)

    # gi stationaries: per (gate g, chunk j) a [2,128] block, both rows =
    # w_ih[g*512 + j*128 : ...]; contract with [tmp; a] rows of S2.
    gilhs = np.broadcast_to(
        w_ih.reshape(3 * KC, 128)[None, :, :], (2, 3 * KC, 128)
    ).reshape(2, 12 * 128).astype(f16).copy()

    bias_regions = [
        b_ih[0:H] + b_hh[0:H],          # r
        b_ih[H:2 * H] + b_hh[H:2 * H],  # z
        b_hh[2 * H:3 * H],              # n: b_hh only
        p_b1,                           # ph
        b_ih[2 * H:3 * H],              # gin: b_ih_n
    ]
    bcols = np.concatenate(
        [br.reshape(KC, 128) for br in bias_regions], axis=1
    ).astype(f16)                                        # [4, 5*128]

    e4 = np.zeros((KC, KC, bl), np.float32)
    for j in range(KC):
        e4[j, j, :] = 1.0
    e4 = e4.reshape(KC, KC * bl).astype(f16)

    pw2t = p_w2[0].reshape(KC, 128).T.astype(f16).copy()
    pb2 = p_b2.reshape(1, 1).astype(np.float32)

    return {
        "wt": wt, "gilhs": gilhs, "bcols": bcols, "e4": e4,
        "pw2t": pw2t, "pb2": pb2,
        "a_arr": a_arr[:t_steps], "m_arr": m_arr[:t_steps],
    }


def _assemble(results, t_steps=T, bl=BL):
    """results: list of 8 per-core dicts with 'outl' [T,128,KC,bl] fp16."""
    out = np.zeros((B, t_steps, 2 * H), np.float32)
    for core in range(NCORES):
        direction = 0 if core < 4 else 1
        bg = core % 4
        arr = np.asarray(results[core]["outl"], np.float16).astype(np.float32)
        # [t, p, j, b] -> [b, t, j, p] -> [b, t, 512]
        arr = arr.transpose(3, 0, 2, 1).reshape(bl, t_steps, H)
        if direction == 1:
            arr = arr[:, ::-1]
        out[bg * bl:(bg + 1) * bl, :, direction * H:(direction + 1) * H] = arr
    return out


def kernel(**inputs):
    from concourse.bass_utils import run_bass_kernel_spmd

    key = (T, U_DEF, BL)
    if key not in _cache:
        _cache[key] = _build_program(T, U_DEF, BL)
    nc = _cache[key]

    in_maps = [_prep_core_inputs(inputs, c) for c in range(NCORES)]
    res = run_bass_kernel_spmd(
        nc, in_maps, core_ids=list(range(NCORES)), trace=False
    )
    return _assemble(res.results)


# revision 8
# speedup vs baseline: 1.0925x; 1.0925x over previous
# kernel.py — Bidirectional masked-GRU-with-predictor on 8 Trainium2 NeuronCores.
#
# Problem (reference.py): B=128, T=1024, H=512
#   per step, per direction:
#     x_in = where(mask, predictor(h), x)            predictor: Linear(H,H)->ReLU->Linear(H,1)->Tanh
#     h    = GRUCell(h, x_in)                        PyTorch gate order (r, z, n)
#   output [B, T, 2H] = concat(fwd hidden states, time-reversed bwd hidden states)
#
# Sharding: 8 cores = 2 directions x 4 batch groups of 32.  All cores run the
# SAME Bass program; per-core data differs (bwd cores get time-reversed x/mask
# and their outputs are flipped back on the host).
#
# On-core layout ("feature-major, chunk-in-free"):
#   h^T kept as [128 partitions = feature%128, (j,b)] where j = feature//128 (4 chunks),
#   b = local batch (32).  Big matmul: stationary = W^T 128x128 blocks (fp16, FWL),
#   moving = h chunks; gates + predictor-hidden land in PSUM feature-major, so the
#   new h is produced directly in the layout the next step's matmul consumes.
#   fp16 matmul inputs + fp32 PSUM accumulate + fp32 vector math:
#   measured emulation error vs fp32 reference: ~7e-4 of output absmax.

import numpy as np

B, T, H = 128, 1024, 512
NCORES = 8
BL = B // 4          # 32: batch per core (4 groups x 2 directions)
KC = H // 128        # 4 contraction chunks
MC = (3 * H + H) // 128  # 16 output chunks (w_hh 12 + p_w1 4)
U_DEF = 32           # time steps per For_i iteration

_cache = {}


def _build_program(t_steps=T, u_steps=U_DEF, bl=BL, n_cores=NCORES):
    import concourse.bacc as bacc
    import concourse.bass as bass
    import concourse.tile as tile
    from concourse.tile import add_dep_helper
    from concourse import mybir

    f16 = mybir.dt.float16
    f32 = mybir.dt.float32

    nc = bacc.Bacc(
        "TRN2",
        target_bir_lowering=False,
        debug=False,
        enable_asserts=False,
        num_devices=n_cores,
    )

    # ---- DRAM tensors (per-core data; same names on every core) ----
    d_wt = nc.dram_tensor("wt", [128, MC * KC * 128], f16, kind="ExternalInput").ap()
    d_gi = nc.dram_tensor("gilhs", [2, 12 * 128], f16, kind="ExternalInput").ap()
    d_bc = nc.dram_tensor("bcols", [4, 5 * 128], f16, kind="ExternalInput").ap()
    d_e4 = nc.dram_tensor("e4", [4, KC * bl], f16, kind="ExternalInput").ap()
    d_pw2 = nc.dram_tensor("pw2t", [128, KC], f16, kind="ExternalInput").ap()
    d_pb2 = nc.dram_tensor("pb2", [1, 1], f32, kind="ExternalInput").ap()
    d_a = nc.dram_tensor("a_arr", [t_steps, bl], f16, kind="ExternalInput").ap()
    d_m = nc.dram_tensor("m_arr", [t_steps, bl], f16, kind="ExternalInput").ap()
    d_out = nc.dram_tensor(
        "outl", [t_steps, 128, KC, bl], f16, kind="ExternalOutput"
    ).ap()

    Relu = mybir.ActivationFunctionType.Relu
    Tanh = mybir.ActivationFunctionType.Tanh
    Sigmoid = mybir.ActivationFunctionType.Sigmoid

    with tile.TileContext(nc) as tc:
        import contextlib

        with contextlib.ExitStack() as ctx:
            consts = ctx.enter_context(tc.tile_pool(name="consts", bufs=1))
            psum = ctx.enter_context(tc.tile_pool(name="psum", bufs=1, space="PSUM"))
            work = ctx.enter_context(tc.tile_pool(name="work", bufs=2))
            io = ctx.enter_context(tc.tile_pool(name="io", bufs=2))

            # ---- constant preload ----
            WT = consts.tile([128, MC * KC * 128], f16, tag="WT")
            GIL = consts.tile([2, 12 * 128], f16, tag="GIL")
            BC = consts.tile([4, 5 * 128], f16, tag="BC")
            E4 = consts.tile([4, KC * bl], f16, tag="E4")
            PW2 = consts.tile([128, KC], f16, tag="PW2")
            PB2 = consts.tile([1, 1], f32, tag="PB2")
            for dst, src in (
                (WT, d_wt), (GIL, d_gi), (BC, d_bc),
                (E4, d_e4), (PW2, d_pw2), (PB2, d_pb2),
            ):
                nc.sync.dma_start(out=dst, in_=src)

            # persistent ping-pong hidden state, fp16, [128, (j,b)]
            h0 = consts.tile([128, KC * bl], f16, tag="h0")
            h1 = consts.tile([128, KC * bl], f16, tag="h1")
            nc.vector.memset(h0, 0.0)
            nc.vector.memset(h1, 0.0)
            h_tiles = [h0, h1]

            # persistent PSUM accumulators (single-buffered; readers drain early)
            G_r = psum.tile([128, KC * bl], f32, tag="G_r")
            G_z = psum.tile([128, KC * bl], f32, tag="G_z")
            G_n = psum.tile([128, KC * bl], f32, tag="G_n")
            PHp = psum.tile([128, KC * bl], f32, tag="PH")
            GIN = psum.tile([128, KC * bl], f32, tag="GIN")
            PREN = psum.tile([128, KC * bl], f32, tag="PREN")
            PRD = psum.tile([1, bl], f32, tag="PRD")

            def w_block(m, k):
                bi = m * KC + k
                return WT[:, bi * 128:(bi + 1) * 128]

            def emit_region(g_idx, region, h_cur, has_gi=False):
                # bias matmul opens the accumulation (start=True covers the
                # whole region), then 4 m-chunks x 4 k-chunks of W blocks.
                # Returns (first, last) instruction for PE-order chaining.
                first = nc.tensor.matmul(
                    region, BC[:, g_idx * 128:(g_idx + 1) * 128], E4,
                    start=True, stop=False, skip_group_check=True,
                )
                base_m = g_idx * KC if g_idx < 3 else 12
                last = first
                for j in range(KC):
                    m = base_m + j
                    for k in range(KC):
                        last = nc.tensor.matmul(
                            region[:, j * bl:(j + 1) * bl],
                            w_block(m, k),
                            h_cur[:, k * bl:(k + 1) * bl],
                            start=False,
                            stop=(not has_gi and k == KC - 1),
                            skip_group_check=True,
                        )
                return first, last

            def emit_gi(g_idx, region, gi_rhs):
                # K=2 rank-1 matmuls: region[:, j] += w_ih_g[j] (x) x_in
                first = last = None
                for j in range(KC):
                    gj = g_idx * KC + j
                    last = nc.tensor.matmul(
                        region[:, j * bl:(j + 1) * bl],
                        GIL[:, gj * 128:(gj + 1) * 128],
                        gi_rhs,
                        start=False, stop=True, skip_group_check=True,
                    )
                    if first is None:
                        first = last
                return first, last

            def pe_order(a_first, b_last):
                # force PE issue order: a runs after b (ordering only)
                add_dep_helper(a_first.ins, b_last.ins, sync=False)

            def step(u, h_cur, h_new, S2, MB, t_dyn):
                gi_rhs = S2[:, u * bl:(u + 1) * bl]
                # PE order: PH, W_r, PRD, W_n, gi_r, GIN, W_z, gi_z
                ph_f, ph_l = emit_region(3, PHp, h_cur)
                r_f, r_l = emit_region(0, G_r, h_cur, has_gi=True)
                pe_order(r_f, ph_l)
                relu = work.tile([128, KC * bl], f16, tag="relu")
                nc.vector.tensor_scalar_max(relu, PHp, 0.0)
                prd_f = prd_l = None
                for k in range(KC):
                    prd_l = nc.tensor.matmul(
                        PRD, PW2[:, k:k + 1], relu[:, k * bl:(k + 1) * bl],
                        start=(k == 0), stop=(k == KC - 1), skip_group_check=True,
                    )
                    if prd_f is None:
                        prd_f = prd_l
                pe_order(prd_f, r_l)
                pred = work.tile([1, bl], f16, tag="pred")
                nc.scalar.activation(out=pred, in_=PRD, func=Tanh, bias=PB2[:, :])
                nc.vector.tensor_mul(
                    S2[0:1, u * bl:(u + 1) * bl], pred, MB[0:1, u * bl:(u + 1) * bl]
                )

                n_f, n_l = emit_region(2, G_n, h_cur)
                pe_order(n_f, prd_l)
                gir_f, gir_l = emit_gi(0, G_r, gi_rhs)
                pe_order(gir_f, n_l)
                gin_f = nc.tensor.matmul(
                    GIN, BC[:, 4 * 128:5 * 128], E4,
                    start=True, stop=False, skip_group_check=True)
                pe_order(gin_f, gir_l)
                _, gin_l = emit_gi(2, GIN, gi_rhs)
                z_f, z_l = emit_region(1, G_z, h_cur, has_gi=True)
                pe_order(z_f, gin_l)
                giz_f, _ = emit_gi(1, G_z, gi_rhs)
                pe_order(giz_f, z_l)

                r_sb = work.tile([128, KC * bl], f16, tag="r_sb")
                nc.scalar.activation(out=r_sb, in_=G_r, func=Sigmoid)

                # n = tanh(GIN + r * G_n)
                u_n = work.tile([128, KC * bl], f32, tag="u_n")
                nc.vector.tensor_mul(u_n, r_sb, G_n)
                nc.vector.tensor_add(PREN, u_n, GIN)
                n_sb = work.tile([128, KC * bl], f16, tag="n_sb")
                nc.scalar.activation(out=n_sb, in_=PREN, func=Tanh)

                z_sb = work.tile([128, KC * bl], f16, tag="z_sb")
                nc.scalar.activation(out=z_sb, in_=G_z, func=Sigmoid)

                # h' = z*h - (z-1)*n  == z*h + (1-z)*n
                t1 = work.tile([128, KC * bl], f16, tag="t1")
                nc.vector.tensor_mul(t1, z_sb, h_cur)
                t2 = work.tile([128, KC * bl], f16, tag="t2")
                nc.vector.scalar_tensor_tensor(
                    out=t2, in0=z_sb, scalar=1.0, in1=n_sb,
                    op0=mybir.AluOpType.subtract, op1=mybir.AluOpType.mult,
                )
                nc.vector.tensor_sub(h_new, t1, t2)

                # stream h' out:  outl[t, p, j, b]
                dst = d_out[bass.ds(t_dyn, 1)].rearrange("o p j b -> (o p) j b")
                nc.sync.dma_start(
                    out=dst, in_=h_new.rearrange("p (j b) -> p j b", b=bl)
                )

            n_blocks = t_steps // u_steps
            with tc.For_i(
                0, n_blocks, 1, hint_engines=(mybir.EngineType.PE,)
            ) as iv:
                S2 = io.tile([2, u_steps * bl], f16, tag="S2")
                MB = io.tile([1, u_steps * bl], f16, tag="MB")
                nc.sync.dma_start(
                    out=S2[1:2, :].rearrange("p (u b) -> p u b", b=bl),
                    in_=d_a[bass.ds(iv * u_steps, u_steps)].unsqueeze(0),
                )
                nc.sync.dma_start(
                    out=MB[0:1, :].rearrange("p (u b) -> p u b", b=bl),
                    in_=d_m[bass.ds(iv * u_steps, u_steps)].unsqueeze(0),
                )
                for u in range(u_steps):
                    step(
                        u,
                        h_tiles[u % 2],
                        h_tiles[(u + 1) % 2],
                        S2,
                        MB,
                        iv * u_steps + u,
                    )

    nc.compile()
    return nc


def _prep_core_inputs(inputs, core, t_steps=T, bl=BL):
    """Build the per-core input map (numpy) for core id `core`."""
    f16 = np.float16
    direction = 0 if core < 4 else 1  # 0 fwd, 1 bwd
    bg = core % 4
    sl = slice(bg * bl, (bg + 1) * bl)

    x = np.asarray(inputs["x"], np.float32)[:, :, 0]      # [B, T]
    msk = np.asarray(inputs["mask"]).astype(np.float32)[:, :, 0]
    pfx = "wf" if direction == 0 else "wb"
    w_ih = np.asarray(inputs[f"{pfx}_ih"], np.float32)[:, 0]   # [3H]
    w_hh = np.asarray(inputs[f"{pfx}_hh"], np.float32)         # [3H, H]
    b_ih = np.asarray(inputs[f"b{pfx[1]}_ih"], np.float32)
    b_hh = np.asarray(inputs[f"b{pfx[1]}_hh"], np.float32)
    p_w1 = np.asarray(inputs["p_w1"], np.float32)
    p_b1 = np.asarray(inputs["p_b1"], np.float32)
    p_w2 = np.asarray(inputs["p_w2"], np.float32)
    p_b2 = np.asarray(inputs["p_b2"], np.float32)

    xs = x[sl].T.copy()      # [T, bl]
    ms = msk[sl].T.copy()
    if direction == 1:
        xs = xs[::-1].copy()
        ms = ms[::-1].copy()
    a_arr = (xs * (1.0 - ms)).astype(f16)
    m_arr = ms.astype(f16)

    W = np.concatenate([w_hh, p_w1], axis=0)             # [2048, 512]
    Wr = W.reshape(MC, 128, KC, 128)                     # [m, c, k, p]
    wt = Wr.transpose(3, 0, 2, 1).reshape(128, MC * KC * 128).astype(f16)

    # gi stationaries: per (gate g, chunk j) a [2,128] block, both rows =
    # w_ih[g*512 + j*128 : ...]; contract with [tmp; a] rows of S2.
    gilhs = np.broadcast_to(
        w_ih.reshape(3 * KC, 128)[None, :, :], (2, 3 * KC, 128)
    ).reshape(2, 12 * 128).astype(f16).copy()

    bias_regions = [
        b_ih[0:H] + b_hh[0:H],          # r
        b_ih[H:2 * H] + b_hh[H:2 * H],  # z
        b_hh[2 * H:3 * H],              # n: b_hh only
        p_b1,                           # ph
        b_ih[2 * H:3 * H],              # gin: b_ih_n
    ]
    bcols = np.concatenate(
        [br.reshape(KC, 128) for br in bias_regions], axis=1
    ).astype(f16)                                        # [4, 5*128]

    e4 = np.zeros((KC, KC, bl), np.float32)
    for j in range(KC):
        e4[j, j, :] = 1.0
    e4 = e4.reshape(KC, KC * bl).astype(f16)

    pw2t = p_w2[0].reshape(KC, 128).T.astype(f16).copy()
    pb2 = p_b2.reshape(1, 1).astype(np.float32)

    return {
        "wt": wt, "gilhs": gilhs, "bcols": bcols, "e4": e4,
        "pw2t": pw2t, "pb2": pb2,
        "a_arr": a_arr[:t_steps], "m_arr": m_arr[:t_steps],
    }


def _assemble(results, t_steps=T, bl=BL):
    """results: list of 8 per-core dicts with 'outl' [T,128,KC,bl] fp16."""
    out = np.zeros((B, t_steps, 2 * H), np.float32)
    for core in range(NCORES):
        direction = 0 if core < 4 else 1
        bg = core % 4
        arr = np.asarray(results[core]["outl"], np.float16).astype(np.float32)
        # [t, p, j, b] -> [b, t, j, p] -> [b, t, 512]
        arr = arr.transpose(3, 0, 2, 1).reshape(bl, t_steps, H)
        if direction == 1:
            arr = arr[:, ::-1]
        out[bg * bl:(bg + 1) * bl, :, direction * H:(direction + 1) * H] = arr
    return out


def kernel(**inputs):
    from concourse.bass_utils import run_bass_kernel_spmd

    key = (T, U_DEF, BL)
    if key not in _cache:
        _cache[key] = _build_program(T, U_DEF, BL)
    nc = _cache[key]

    in_maps = [_prep_core_inputs(inputs, c) for c in range(NCORES)]
    res = run_bass_kernel_spmd(
        nc, in_maps, core_ids=list(range(NCORES)), trace=False
    )
    return _assemble(res.results)
